# revision 1
# baseline (speedup 1.0000x reference)
"""DeepTEN encoding kernel for Trainium2 (8 NeuronCores, SPMD data-parallel over batch).

Math (per batch b):
    xf = x[b] viewed (D, N), N = H*W
    dist[n,k] = ||xf[:,n] - c[k]||^2 ;  logits = -scale * dist ;  A = softmax_k(logits)
    E[k,d] = sum_n A[n,k] * (xf[d,n] - c[k,d]) = (A^T X)[k,d] - colsum(A)[k]*c[k,d]

Device decomposition (softmax in (n-partitions, k-free) layout):
    w = -scale (>0), maxs = max(w)
    PSUM(xc)[n,k] = -2*w_k*<x_n,c_k> + w_k*csq_k      (x-tile-stationary matmuls +
                                                       a rank-1 seed matmul)
    G[n,k] = exp((w_k-maxs)*x_sq[n])                  (DVE mult + ACT exp; x_sq is
                                                       computed exactly on host, fp32)
    P = exp(PSUM) * G ;  S[n] = sum_k P ;  A = P / S
    (shifting logits by maxs*x_sq[n] bounds exp args; the gap to the true rowmax
     is < ~4 so S never underflows)
    psum_E[k, 0:D] += sum_n A[n,k]*xT[n,d]            (PE accumulates whole batch,
    psum_E[k, D]   += sum_n A[n,k]                     ones-column fused colsum)

x is uploaded twice in bf16 — (D,N) for the distance matmuls and pre-transposed
tiles (p, gi, d) for the aggregation matmuls — so no on-device transpose is needed;
total HBM traffic ~= one fp32 read of x. The mm2s of superblock s are emitted after
the softmax chain of superblock s+1 (software pipelining) and ping-pong between two
PSUM accumulators to avoid back-to-back accumulate stalls. Host does only layout /
dtype transforms of inputs plus the tiny (K,D)-level finishing arithmetic.
"""
import os
import sys
import numpy as np

sys.path.insert(0, "/opt/trn_rl_repo")

import ml_dtypes  # noqa: E402

BF16 = ml_dtypes.bfloat16

B, D, H, W = 32, 128, 128, 128
K = 32
N = H * W            # 16384
NCORES = 8
BPC = B // NCORES    # batches per core
TILN = 128           # n per tile (matmul stationary width)
NTIL = 16            # tiles per block
BLKN = TILN * NTIL   # 2048 n per block
NBLK = N // BLKN     # 8 blocks per batch

_CACHE = {}


def _build_module():
    from contextlib import ExitStack
    import concourse.tile as tile
    from concourse import bacc, mybir

    nc = bacc.Bacc("TRN2", target_bir_lowering=False, debug=False, num_devices=NCORES)
    bf = mybir.dt.bfloat16
    f32 = mybir.dt.float32

    x_d = nc.dram_tensor("x", [BPC, D, N], bf, kind="ExternalInput").ap()
    # xt[b, p, gi, d] = x[b, d, gi*128 + p]
    xt_d = nc.dram_tensor("xt", [BPC, 128, N // TILN, D + 1], bf, kind="ExternalInput").ap()
    # xsqc[b, p, sib, j] = x_sq[b, n],  n = sib*4096 + j*128 + p
    xsqc_d = nc.dram_tensor("xsqc", [BPC, 128, N // 4096, 32], f32, kind="ExternalInput").ap()
    # wmrep[p, j*K+k] = w[k] - maxs   (f32: bf16 would lose ~0.25 abs on wm*xsq)
    wmrep_d = nc.dram_tensor("wmrep", [128, 32 * K], f32, kind="ExternalInput").ap()
    # wcsqrep[0, j*K+k] = w[k]*csq[k]
    wcsqr_d = nc.dram_tensor("wcsqrep", [1, 32 * K], bf, kind="ExternalInput").ap()
    w1_d = nc.dram_tensor("w1", [D, K], bf, kind="ExternalInput").ap()
    oute_d = nc.dram_tensor("out_e", [BPC, K, 2, D + 1], f32, kind="ExternalOutput").ap()

    with tile.TileContext(nc) as tc, ExitStack() as ctx:
        cpool = ctx.enter_context(tc.tile_pool(name="const", bufs=1))
        xpool = ctx.enter_context(tc.tile_pool(name="xblk", bufs=4))
        xtpool = ctx.enter_context(tc.tile_pool(name="xtblk", bufs=4))
        qpool = ctx.enter_context(tc.tile_pool(name="xsqb", bufs=2))
        ppool = ctx.enter_context(tc.tile_pool(name="pexp", bufs=3))
        npool = ctx.enter_context(tc.tile_pool(name="pnorm", bufs=3))
        vpool = ctx.enter_context(tc.tile_pool(name="small", bufs=4))
        ps_xc = ctx.enter_context(tc.tile_pool(name="ps_xc", bufs=2, space="PSUM"))
        ps_e = ctx.enter_context(tc.tile_pool(name="ps_e", bufs=2, space="PSUM"))

        w1_sb = cpool.tile([D, K], bf)
        nc.sync.dma_start(out=w1_sb[:], in_=w1_d[:, :])
        wmrep_sb = cpool.tile([128, 32 * K], f32)
        nc.sync.dma_start(out=wmrep_sb[:], in_=wmrep_d[:, :])
        wcsqr_sb = cpool.tile([1, 32 * K], bf)
        nc.sync.dma_start(out=wcsqr_sb[:], in_=wcsqr_d[:, :])
        ones1_sb = cpool.tile([1, 128], bf)
        nc.vector.memset(ones1_sb[:], 1.0)

        NSUP = 2                 # blocks per superblock load
        SUPN = BLKN * NSUP       # 4096 n per load chunk
        NSB = NBLK // NSUP       # superblocks per batch
        TPS = NTIL * NSUP        # 32 tiles per superblock

        # Software pipeline: mm2s of superblock s are emitted after the
        # softmax chain of superblock s+1, so the PE hides the chain latency.
        pending = []  # (b, sup_in_batch, pn_sb, xt_sb)
        psum_es = {}
        g_bs = {}
        first_mm2 = {}

        def emit_mm2s(b, sib, pn_sb, xt_sb):
            pe0, pe1 = psum_es[b]
            ff = first_mm2[b]
            for i in range(TPS):
                pp = i % 2
                nc.tensor.matmul(
                    (pe0, pe1)[pp][:],
                    lhsT=pn_sb[:, K * i : K * (i + 1)],
                    rhs=xt_sb[:, i, :],
                    start=ff[pp],
                    stop=(sib == NSB - 1 and i >= TPS - 2),
                )
                ff[pp] = False
            if sib == NSB - 1:
                e_sb = vpool.tile([K, 2, D + 1], f32, tag="e_out")
                nc.vector.tensor_copy(e_sb[:, 0, :], pe0[:])
                nc.vector.tensor_copy(e_sb[:, 1, :], pe1[:])
                nc.sync.dma_start(out=oute_d[b], in_=e_sb[:])

        def emit_prologue(b, sliced):
            """Load xsq and build G = exp(wm*x_sq) for batch b. For batch 0
            (`sliced`), build G in per-superblock slices so the first consumer
            starts ~6us sooner; later batches are emitted one batch early so
            the big DVE/ACT ops land where those engines have slack."""
            xsq_b = qpool.tile(
                [128, (N // 4096) * 32], f32, name=f"xsq_b{b}", tag="xsqb"
            )
            nc.sync.dma_start(
                out=xsq_b[:], in_=xsqc_d[b].rearrange("p s j -> p (s j)")
            )
            t1_b = qpool.tile([128, N // 128 * K], f32, name=f"t1_b{b}", tag="t1b")
            g_b = qpool.tile([128, N // 128 * K], bf, name=f"g_b{b}", tag="gb")
            if sliced:
                for s in range(NSB):
                    sl = slice(s * TPS * K, (s + 1) * TPS * K)
                    nc.vector.tensor_tensor(
                        t1_b[:, sl].rearrange("p (j k) -> p j k", k=K),
                        wmrep_sb[:].rearrange("p (j k) -> p j k", k=K),
                        xsq_b[:, s * TPS : (s + 1) * TPS]
                        .broadcast_to([128, TPS, K]),
                        op=mybir.AluOpType.mult,
                    )
                    nc.scalar.activation(
                        g_b[:, sl], t1_b[:, sl], mybir.ActivationFunctionType.Exp
                    )
            else:
                nc.vector.tensor_tensor(
                    t1_b[:].rearrange("p (s j k) -> p s j k", j=32, k=K),
                    wmrep_sb[:].rearrange("p (j k) -> p j k", k=K)[:, None, :, :]
                    .broadcast_to([128, NBLK // NSUP, 32, K]),
                    xsq_b[:].rearrange("p (s j) -> p s j", j=32)[:, :, :, None]
                    .broadcast_to([128, NBLK // NSUP, 32, K]),
                    op=mybir.AluOpType.mult,
                )
                nc.scalar.activation(
                    g_b[:], t1_b[:], mybir.ActivationFunctionType.Exp
                )
            g_bs[b] = g_b

        for gsup in range(BPC * NSB):
            b, sib = divmod(gsup, NSB)
            if sib == 0:
                emit_prologue(b, sliced=(b == 0))
                psum_es[b] = (
                    ps_e.tile([K, D + 1], f32, tag="pe0", name=f"psum_e0_b{b}"),
                    ps_e.tile([K, D + 1], f32, tag="pe1", name=f"psum_e1_b{b}"),
                )
                first_mm2[b] = [True, True]
            soff = sib * SUPN
            x_sb = xpool.tile([D, SUPN], bf)
            nc.sync.dma_start(out=x_sb[:], in_=x_d[b][:, soff : soff + SUPN])
            xt_sb = xtpool.tile([128, TPS, D + 1], bf)
            nc.scalar.dma_start(
                out=xt_sb[:], in_=xt_d[b][:, sib * TPS : (sib + 1) * TPS, :]
            )
            psum_xc = ps_xc.tile([128, TPS * K], f32)
            for h in range(2):
                nc.tensor.matmul(
                    psum_xc[:, 512 * h : 512 * (h + 1)],
                    lhsT=ones1_sb[:],
                    rhs=wcsqr_sb[:, 512 * h : 512 * (h + 1)],
                    start=True,
                    stop=False,
                    skip_group_check=True,
                )
            for i in range(TPS):
                nc.tensor.matmul(
                    psum_xc[:, K * i : K * (i + 1)],
                    lhsT=x_sb[:, TILN * i : TILN * (i + 1)],
                    rhs=w1_sb[:, :],
                    start=False,
                    stop=True,
                    skip_group_check=True,
                )

            pe_sb = ppool.tile([128, TPS * K], bf, tag="pexp")
            nc.scalar.activation(
                pe_sb[:], psum_xc[:], mybir.ActivationFunctionType.Exp
            )
            p_sb = ppool.tile([128, TPS * K], bf, tag="p")
            nc.gpsimd.tensor_mul(
                p_sb[:],
                pe_sb[:],
                g_bs[b][:, sib * TPS * K : (sib + 1) * TPS * K],
            )
            p3 = p_sb[:].rearrange("p (i k) -> p i k", k=K)
            s_sb = vpool.tile([128, TPS], f32, tag="s")
            nc.vector.reduce_sum(s_sb[:], p3, axis=mybir.AxisListType.X)
            sinv_sb = vpool.tile([128, TPS], f32, tag="sinv")
            nc.vector.reciprocal(sinv_sb[:], s_sb[:])
            pn_sb = npool.tile([128, TPS * K], bf, tag="pn")
            nc.vector.tensor_tensor(
                pn_sb[:].rearrange("p (i k) -> p i k", k=K),
                p3,
                sinv_sb[:].broadcast_to([128, TPS, K]),
                op=mybir.AluOpType.mult,
            )

            pending.append((b, sib, pn_sb, xt_sb))
            if len(pending) > 1:
                emit_mm2s(*pending.pop(0))

        while pending:
            emit_mm2s(*pending.pop(0))

    nc.compile()
    return nc


def _get_module():
    if "nc" not in _CACHE:
        _CACHE["nc"] = _build_module()
    return _CACHE["nc"]


def _host_prep(x, codewords, scale):
    x = np.asarray(x, dtype=np.float32)
    c = np.asarray(codewords, dtype=np.float32)
    s = np.asarray(scale, dtype=np.float32)

    w = -s                           # (K,) in (0, 1)
    maxs = float(w.max())
    w1 = (-2.0 * (w[:, None] * c)).T.astype(BF16)           # (D, K)
    wm = w - maxs                                           # (K,) <= 0
    wcsq = w * (c * c).sum(axis=1)                          # (K,)

    xf = x.reshape(B, D, N)
    xsq = np.einsum("bdn,bdn->bn", xf, xf)                  # (B, N) fp32
    # xsqc[b, p, s, j] = xsq[b, s*4096 + j*128 + p]
    xsqc = np.ascontiguousarray(
        xsq.reshape(B, N // 4096, 32, 128).transpose(0, 3, 1, 2)
    )                                                       # (B, 128, N/4096, 32) f32
    wmrep = np.ascontiguousarray(
        np.broadcast_to(np.tile(wm, 32)[None, :], (128, 32 * K))
    ).astype(np.float32)
    wcsqrep = np.tile(wcsq, 32)[None, :].astype(BF16)       # (1, 32*K)

    xb = xf.astype(BF16)                                    # (B, D, N)
    # xt[b, p, gi, d] = xf[b, d, gi*128 + p];  xt[..., D] = 1.0 (fused colsum column)
    xt = np.ones((B, N // TILN, TILN, D + 1), dtype=BF16)
    xt[:, :, :, :D] = xf.transpose(0, 2, 1).reshape(B, N // TILN, TILN, D).astype(BF16)
    xt = np.ascontiguousarray(xt.transpose(0, 2, 1, 3))     # (B, 128, N/128, D+1)
    return xb, xt, xsqc, wmrep, wcsqrep, w1


def make_in_maps(x, codewords, scale):
    xb, xt, xsqc, wmrep, wcsqrep, w1 = _host_prep(x, codewords, scale)
    in_maps = []
    for ci in range(NCORES):
        sl = slice(BPC * ci, BPC * (ci + 1))
        in_maps.append(
            {
                "x": np.ascontiguousarray(xb[sl]),
                "xt": np.ascontiguousarray(xt[sl]),
                "xsqc": np.ascontiguousarray(xsqc[sl]),
                "wmrep": wmrep,
                "wcsqrep": wcsqrep,
                "w1": w1,
            }
        )
    return in_maps


def finish_output(results, codewords):
    c = np.asarray(codewords, dtype=np.float32)
    out = np.zeros((B, K * D), dtype=np.float32)
    for ci, r in enumerate(results):
        for bb in range(BPC):
            e_parts = r["out_e"][bb][:, 0, :] + r["out_e"][bb][:, 1, :]   # (K, D+1)
            e = e_parts[:, :D] - e_parts[:, D : D + 1] * c
            out[BPC * ci + bb] = e.reshape(-1)
    return out


def kernel(x, codewords, scale):
    from concourse.bass_utils import run_bass_kernel_spmd
    from concourse.bass_interp import get_hw_module

    nc = _get_module()
    in_maps = make_in_maps(x, codewords, scale)

    old_m = nc.m
    nc.m = get_hw_module(nc.m)
    try:
        res = run_bass_kernel_spmd(nc, in_maps, core_ids=list(range(NCORES)))
    finally:
        nc.m = old_m
    return finish_output(res.results, codewords)



# revision 3
# speedup vs baseline: 1.1699x; 1.1699x over previous
"""DeepTEN encoding kernel for Trainium2 (8 NeuronCores, SPMD data-parallel over batch).

Math (per batch b):
    xf = x[b] viewed (D, N), N = H*W
    logits[n,k] = w_k * dist[n,k],  w = -scale > 0 ;  A = softmax_k(logits)
    E[k,d] = sum_n A[n,k] * (xf[d,n] - c[k,d]) = (A^T X)[k,d] - colsum(A)[k]*c[k,d]

Device decomposition (softmax in (n-partitions, k-free) layout, PSUM holds the
FULL shifted exponent so a single EXP activation yields the softmax numerator):
    PSUM[n,(j,k)]  = seed + mm1, everything prescaled by 64:
      seed (one [97,128]x[97,1024] bf16 matmul per superblock) adds
        64*[ (w_k-maxs)*xsq_n + w_k*csq_k ]
      using an exact hi/lo split: wm*xsq ~= wh*m + wh*r + wl*m with
        wh=bf16(wm), wl=bf16(wm-wh), m=bf16(xsq), r=bf16(xsq-m)
      laid out as a block-diagonal rhs over the 32 n-subtiles j'.
      mm1 (fp8 e3m4 x-tile stationary, fp8 w1=-128*w*c moving) adds 64*(-2w<x,c>).
    P = exp(PSUM/64) via one ACT instruction (scale=1/64), S = rowsum_k, A = P/S.
    psum_E[k, 0:D] += sum_n A[n,k]*xT[n,d]   (PE accumulates whole batch,
    psum_E[k, D]   += sum_n A[n,k]            ones-column fused colsum)

x is uploaded twice in fp8 e3m4 — (D,N) for the distance matmuls and
pre-transposed tiles (p, gi, d) for the aggregation matmuls — so total HBM
traffic ~= 17.6 MB/core (vs 33.5 MB for the bf16 dual upload). e3m4 keeps
4 mantissa bits; numpy sim of this exact quantization gives maxabsrel ~9e-3
vs the 2e-2 gate. A stays bf16 (fp8 A fails the gate). The mm2s of
superblock s are emitted after the softmax chain of superblock s+1
(software pipelining) and ping-pong between two PSUM accumulators.
"""
import os
import sys
import numpy as np

sys.path.insert(0, "/opt/trn_rl_repo")

import ml_dtypes  # noqa: E402

BF16 = ml_dtypes.bfloat16
F8E3 = ml_dtypes.float8_e3m4

B, D, H, W = 32, 128, 128, 128
K = 32
N = H * W            # 16384
NCORES = 8
BPC = B // NCORES    # batches per core
TILN = 128           # n per tile (matmul stationary width)
TPS = 32             # tiles per superblock
SUPN = TILN * TPS    # 4096 n per superblock
NSB = N // SUPN      # 4 superblocks per batch
SEEDR = 97           # seed lhsT rows: 32 m + 32 r + 32 m + 1 ones

_CACHE = {}


def _build_module():
    from contextlib import ExitStack
    import concourse.tile as tile
    from concourse import bacc, mybir

    nc = bacc.Bacc("TRN2", target_bir_lowering=False, debug=False, num_devices=NCORES)
    bf = mybir.dt.bfloat16
    f8 = mybir.dt.float8e3
    f32 = mybir.dt.float32

    x_d = nc.dram_tensor("x", [BPC, D, N], f8, kind="ExternalInput").ap()
    # xt[b, p, gi, d] = x[b, d, gi*128 + p]; xt[..., D] = 1.0 (fused colsum col)
    xt_d = nc.dram_tensor("xt", [BPC, 128, N // TILN, D + 1], f8, kind="ExternalInput").ap()
    # seed lhsT rows (see module docstring); xsql[b, row, sb, p]
    xsql_d = nc.dram_tensor("xsql", [BPC, SEEDR, NSB, 128], bf, kind="ExternalInput").ap()
    # seed rhs: wdiag[row, (j, k)] block-diagonal over n-subtiles j
    wdiag_d = nc.dram_tensor("wdiag", [SEEDR, TPS * K], bf, kind="ExternalInput").ap()
    w1_d = nc.dram_tensor("w1", [D, K], f8, kind="ExternalInput").ap()
    oute_d = nc.dram_tensor("out_e", [BPC, K, 2, D + 1], f32, kind="ExternalOutput").ap()

    with tile.TileContext(nc) as tc, ExitStack() as ctx:
        cpool = ctx.enter_context(tc.tile_pool(name="const", bufs=1))
        xpool = ctx.enter_context(tc.tile_pool(name="xblk", bufs=6))
        xtpool = ctx.enter_context(tc.tile_pool(name="xtblk", bufs=6))
        qpool = ctx.enter_context(tc.tile_pool(name="xsqb", bufs=2))
        ppool = ctx.enter_context(tc.tile_pool(name="pexp", bufs=3))
        npool = ctx.enter_context(tc.tile_pool(name="pnorm", bufs=3))
        vpool = ctx.enter_context(tc.tile_pool(name="small", bufs=4))
        ps_xc = ctx.enter_context(tc.tile_pool(name="ps_xc", bufs=2, space="PSUM"))
        ps_e = ctx.enter_context(tc.tile_pool(name="ps_e", bufs=2, space="PSUM"))

        w1_sb = cpool.tile([D, K], f8)
        nc.sync.dma_start(out=w1_sb[:], in_=w1_d[:, :])
        wdiag_sb = cpool.tile([SEEDR, TPS * K], bf)
        nc.sync.dma_start(out=wdiag_sb[:], in_=wdiag_d[:, :])

        # Software pipeline: mm2s of superblock s are emitted after the
        # softmax chain of superblock s+1, so the PE hides the chain latency.
        pending = []  # (b, sib, pn_sb, xt_sb)
        psum_es = {}
        xsql_bs = {}
        first_mm2 = {}

        def emit_mm2s(b, sib, pn_sb, xt_sb):
            pe0, pe1 = psum_es[b]
            ff = first_mm2[b]
            for i in range(TPS):
                pp = i % 2
                nc.tensor.matmul(
                    (pe0, pe1)[pp][:],
                    lhsT=pn_sb[:, K * i : K * (i + 1)],
                    rhs=xt_sb[:, i, :],
                    start=ff[pp],
                    stop=(sib == NSB - 1 and i >= TPS - 2),
                )
                ff[pp] = False
            if sib == NSB - 1:
                e_sb = vpool.tile([K, 2, D + 1], f32, tag="e_out")
                nc.vector.tensor_copy(e_sb[:, 0, :], pe0[:])
                nc.vector.tensor_copy(e_sb[:, 1, :], pe1[:])
                nc.sync.dma_start(out=oute_d[b], in_=e_sb[:])

        for gsup in range(BPC * NSB):
            b, sib = divmod(gsup, NSB)
            if sib == 0:
                xsql_b = qpool.tile([SEEDR, NSB, 128], bf, tag="xsql")
                nc.sync.dma_start(out=xsql_b[:], in_=xsql_d[b])
                xsql_bs[b] = xsql_b
                psum_es[b] = (
                    ps_e.tile([K, D + 1], f32, tag="pe0", name=f"psum_e0_b{b}"),
                    ps_e.tile([K, D + 1], f32, tag="pe1", name=f"psum_e1_b{b}"),
                )
                first_mm2[b] = [True, True]
            soff = sib * SUPN
            x_sb = xpool.tile([D, SUPN], f8)
            nc.sync.dma_start(out=x_sb[:], in_=x_d[b][:, soff : soff + SUPN])
            xt_sb = xtpool.tile([128, TPS, D + 1], f8)
            nc.scalar.dma_start(
                out=xt_sb[:], in_=xt_d[b][:, sib * TPS : (sib + 1) * TPS, :]
            )
            psum_xc = ps_xc.tile([128, TPS * K], f32)
            for h in range(2):
                nc.tensor.matmul(
                    psum_xc[:, 512 * h : 512 * (h + 1)],
                    lhsT=xsql_bs[b][:, sib, :],
                    rhs=wdiag_sb[:, 512 * h : 512 * (h + 1)],
                    start=True,
                    stop=False,
                    skip_group_check=True,
                )
            for i in range(TPS):
                nc.tensor.matmul(
                    psum_xc[:, K * i : K * (i + 1)],
                    lhsT=x_sb[:, TILN * i : TILN * (i + 1)],
                    rhs=w1_sb[:, :],
                    start=False,
                    stop=True,
                    skip_group_check=True,
                )

            p_sb = ppool.tile([128, TPS * K], bf, tag="pexp")
            nc.scalar.activation(
                p_sb[:], psum_xc[:], mybir.ActivationFunctionType.Exp,
                scale=1.0 / 64.0,
            )
            p3 = p_sb[:].rearrange("p (i k) -> p i k", k=K)
            s_sb = vpool.tile([128, TPS], f32, tag="s")
            nc.vector.reduce_sum(s_sb[:], p3, axis=mybir.AxisListType.X)
            sinv_sb = vpool.tile([128, TPS], f32, tag="sinv")
            nc.vector.reciprocal(sinv_sb[:], s_sb[:])
            pn_sb = npool.tile([128, TPS * K], bf, tag="pn")
            norm_eng = nc.vector if sib == 0 else nc.gpsimd
            norm_eng.tensor_tensor(
                pn_sb[:].rearrange("p (i k) -> p i k", k=K),
                p3,
                sinv_sb[:].broadcast_to([128, TPS, K]),
                op=mybir.AluOpType.mult,
            )

            pending.append((b, sib, pn_sb, xt_sb))
            if len(pending) > 1:
                emit_mm2s(*pending.pop(0))

        while pending:
            emit_mm2s(*pending.pop(0))

    nc.compile()
    return nc


def _get_module():
    if "nc" not in _CACHE:
        _CACHE["nc"] = _build_module()
    return _CACHE["nc"]


def _host_prep(x, codewords, scale):
    x = np.asarray(x, dtype=np.float32)
    c = np.asarray(codewords, dtype=np.float32)
    s = np.asarray(scale, dtype=np.float32)

    w = -s                           # (K,) in (0, 1)
    maxs = float(w.max())
    wm = w - maxs                                           # (K,) <= 0
    wh = wm.astype(BF16).astype(np.float32)
    wl = (wm - wh).astype(BF16).astype(np.float32)
    wcsq = w * (c * c).sum(axis=1)                          # (K,)
    w1 = (-128.0 * (w[:, None] * c)).T.astype(F8E3)         # (D, K) = 64 * (-2 w c)

    xf = x.reshape(B, D, N)
    xsq = np.einsum("bdn,bdn->bn", xf, xf)                  # (B, N) fp32
    m = xsq.astype(BF16).astype(np.float32)
    r = (xsq - m).astype(BF16).astype(np.float32)
    # xsql[b, row, sb, p]; row j' in 0..31 -> m of n-subtile j', 32..63 -> r,
    # 64..95 -> m again (for the wl term), 96 -> ones (wcsq term)
    # n = sb*SUPN + j'*128 + p
    mt = m.reshape(B, NSB, TPS, 128).transpose(0, 2, 1, 3)  # (B, j', sb, p)
    rt = r.reshape(B, NSB, TPS, 128).transpose(0, 2, 1, 3)
    xsql = np.empty((B, SEEDR, NSB, 128), dtype=BF16)
    xsql[:, 0:32] = mt.astype(BF16)
    xsql[:, 32:64] = rt.astype(BF16)
    xsql[:, 64:96] = mt.astype(BF16)
    xsql[:, 96] = 1.0

    wdiag = np.zeros((SEEDR, TPS, K), dtype=np.float32)
    jj = np.arange(TPS)
    wdiag[jj, jj, :] = 64.0 * wh[None, :]
    wdiag[32 + jj, jj, :] = 64.0 * wh[None, :]
    wdiag[64 + jj, jj, :] = 64.0 * wl[None, :]
    wdiag[96, :, :] = 64.0 * wcsq[None, :]
    wdiag = wdiag.reshape(SEEDR, TPS * K).astype(BF16)

    x8 = xf.astype(F8E3)                                    # (B, D, N)
    # xt[b, p, gi, d] = xf[b, d, gi*128 + p];  xt[..., D] = 1.0
    xt = np.ones((B, N // TILN, TILN, D + 1), dtype=F8E3)
    xt[:, :, :, :D] = xf.transpose(0, 2, 1).reshape(B, N // TILN, TILN, D).astype(F8E3)
    xt = np.ascontiguousarray(xt.transpose(0, 2, 1, 3))     # (B, 128, N/128, D+1)
    return x8, xt, xsql, wdiag, w1


def make_in_maps(x, codewords, scale):
    x8, xt, xsql, wdiag, w1 = _host_prep(x, codewords, scale)
    in_maps = []
    for ci in range(NCORES):
        sl = slice(BPC * ci, BPC * (ci + 1))
        in_maps.append(
            {
                "x": np.ascontiguousarray(x8[sl]),
                "xt": np.ascontiguousarray(xt[sl]),
                "xsql": np.ascontiguousarray(xsql[sl]),
                "wdiag": wdiag,
                "w1": w1,
            }
        )
    return in_maps


def finish_output(results, codewords):
    c = np.asarray(codewords, dtype=np.float32)
    out = np.zeros((B, K * D), dtype=np.float32)
    for ci, r in enumerate(results):
        for bb in range(BPC):
            e_parts = r["out_e"][bb][:, 0, :] + r["out_e"][bb][:, 1, :]   # (K, D+1)
            e = e_parts[:, :D] - e_parts[:, D : D + 1] * c
            out[BPC * ci + bb] = e.reshape(-1)
    return out


def kernel(x, codewords, scale):
    from concourse.bass_utils import run_bass_kernel_spmd
    from concourse.bass_interp import get_hw_module

    nc = _get_module()
    in_maps = make_in_maps(x, codewords, scale)

    old_m = nc.m
    nc.m = get_hw_module(nc.m)
    try:
        res = run_bass_kernel_spmd(nc, in_maps, core_ids=list(range(NCORES)))
    finally:
        nc.m = old_m
    return finish_output(res.results, codewords)


# revision 6
# speedup vs baseline: 1.2301x; 1.0515x over previous
"""DeepTEN encoding kernel for Trainium2 (8 NeuronCores, SPMD data-parallel over batch).

Math (per batch b):
    xf = x[b] viewed (D, N), N = H*W
    logits[n,k] = w_k * dist[n,k],  w = -scale > 0 ;  A = softmax_k(logits)
    E[k,d] = sum_n A[n,k] * (xf[d,n] - c[k,d]) = (A^T X)[k,d] - colsum(A)[k]*c[k,d]

Device decomposition (softmax in (n-partitions, k-free) layout, PSUM holds the
FULL shifted exponent so a single EXP activation yields the softmax numerator):
    PSUM[n,(j,k)]  = seed + mm1, everything prescaled by 64:
      seed (one [97,128]x[97,1024] bf16 matmul per superblock, split in 2 PSUM
      banks) adds 64*[ (w_k-maxs)*xsq_n + w_k*csq_k ] using an exact hi/lo
      split: wm*xsq ~= wh*m + wh*r + wl*m with wh=bf16(wm), wl=bf16(wm-wh),
      m=bf16(xsq), r=bf16(xsq-m), laid out block-diagonally over n-subtiles j'.
      mm1 (fp8 e3m4 x-tile stationary, fp8 w1=-128*w*c moving) adds 64*(-2w<x,c>).
    P = exp(PSUM/64) via one ACT instruction (scale=1/64), S = rowsum_k, A = P/S.
    psum_E[k, 0:D] += sum_n A[n,k]*xT[n,d]   (PE accumulates whole batch over a
    psum_E[k, D]   += sum_n A[n,k]            4-deep accumulator rotation,
                                              ones-column fused colsum)

x is uploaded twice in fp8 e3m4 — (D,N) for the distance matmuls and
pre-transposed tiles (p, gi, d) for the aggregation matmuls — so total HBM
traffic ~= 17.6 MB/core (vs 33.5 MB for the bf16 dual upload). e3m4 keeps
4 mantissa bits; numpy sim of this exact quantization gives maxabsrel ~1.1e-2
vs the 2e-2 gate. A stays bf16 (fp8 A fails the gate). The mm2s of
superblock s are emitted after the softmax chain of superblock s+1
(software pipelining). Head-latency details: consts + xsql go on the vector
DMA ring so the sync ring's first descriptors are the x chunks the first
matmuls need; x superblocks are loaded in 4 chunks so mm1 starts after ~1/4
of the first superblock has landed.
"""
import os
import sys
import numpy as np

sys.path.insert(0, "/opt/trn_rl_repo")

import ml_dtypes  # noqa: E402

BF16 = ml_dtypes.bfloat16
F8E3 = ml_dtypes.float8_e3m4

B, D, H, W = 32, 128, 128, 128
K = 32
N = H * W            # 16384
NCORES = 8
BPC = B // NCORES    # batches per core
TILN = 128           # n per tile (matmul stationary width)
TPS = 32             # tiles per superblock
SUPN = TILN * TPS    # 4096 n per superblock
NSB = N // SUPN      # 4 superblocks per batch
NCH = 4              # x superblock load chunks
CHN = SUPN // NCH    # 1024 n per chunk
SEEDR = 97           # seed lhsT rows: 32 m + 32 r + 32 m + 1 ones
NACC = 4             # psum_E accumulator rotation depth

_CACHE = {}


def _build_module():
    from contextlib import ExitStack
    import concourse.tile as tile
    from concourse import bacc, mybir

    nc = bacc.Bacc("TRN2", target_bir_lowering=False, debug=False, num_devices=NCORES)
    bf = mybir.dt.bfloat16
    f8 = mybir.dt.float8e3
    f32 = mybir.dt.float32

    x_d = nc.dram_tensor("x", [BPC, D, N], f8, kind="ExternalInput").ap()
    # xt[b, p, gi, d] = x[b, d, gi*128 + p]; xt[..., D] = 1.0 (fused colsum col)
    xt_d = nc.dram_tensor("xt", [BPC, 128, N // TILN, D + 1], f8, kind="ExternalInput").ap()
    # seed lhsT rows (see module docstring); xsql[b, row, sb, p], rows 97..127 pad
    xsql_d = nc.dram_tensor("xsql", [BPC, 128, NSB, 128], bf, kind="ExternalInput").ap()
    # seed rhs: wdiag[row, (j, k)] block-diagonal over n-subtiles j
    wdiag_d = nc.dram_tensor("wdiag", [SEEDR, TPS * K], bf, kind="ExternalInput").ap()
    w1_d = nc.dram_tensor("w1", [D, K], f8, kind="ExternalInput").ap()
    oute_d = nc.dram_tensor("out_e", [BPC, K, NACC, D + 1], f32, kind="ExternalOutput").ap()

    with tile.TileContext(nc) as tc, ExitStack() as ctx:
        cpool = ctx.enter_context(tc.tile_pool(name="const", bufs=1))
        xpool = ctx.enter_context(tc.tile_pool(name="xblk", bufs=3))
        xtpool = ctx.enter_context(tc.tile_pool(name="xtblk", bufs=3))
        qpool = ctx.enter_context(tc.tile_pool(name="xsqb", bufs=2))
        ppool = ctx.enter_context(tc.tile_pool(name="pexp", bufs=3))
        npool = ctx.enter_context(tc.tile_pool(name="pnorm", bufs=3))
        vpool = ctx.enter_context(tc.tile_pool(name="small", bufs=4))
        ps_xc = ctx.enter_context(tc.tile_pool(name="ps_xc", bufs=2, space="PSUM"))
        ps_e = ctx.enter_context(tc.tile_pool(name="ps_e", bufs=1, space="PSUM"))

        w1_sb = cpool.tile([D, K], f8)
        nc.scalar.dma_start(out=w1_sb[:], in_=w1_d[:, :])
        wdiag_sb = cpool.tile([SEEDR, TPS * K], bf)
        nc.scalar.dma_start(out=wdiag_sb[:], in_=wdiag_d[:, :])

        # Software pipeline: mm2s of superblock s are emitted after the
        # softmax chain of superblock s+1, so the PE hides the chain latency.
        pending = []  # (b, sib, pn_sb, xt_sb)
        psum_es = {}
        xsql_bs = {}
        first_mm2 = {}

        def emit_mm2s(b, sib, pn_sb, xt_sb):
            pes = psum_es[b]
            ff = first_mm2[b]
            for i in range(TPS):
                pp = i % NACC
                nc.tensor.matmul(
                    pes[pp][:],
                    lhsT=pn_sb[:, K * i : K * (i + 1)],
                    rhs=xt_sb[:, i, :],
                    start=ff[pp],
                    stop=(sib == NSB - 1 and i >= TPS - NACC),
                )
                ff[pp] = False
            if sib == NSB - 1:
                e_sb = vpool.tile([K, NACC, D + 1], f32, tag="e_out")
                for pp in range(NACC):
                    nc.vector.tensor_copy(e_sb[:, pp, :], pes[pp][:])
                nc.sync.dma_start(out=oute_d[b], in_=e_sb[:])

        for gsup in range(BPC * NSB):
            b, sib = divmod(gsup, NSB)
            if sib == 0:
                xsql_b = qpool.tile([128, NSB, 128], bf, tag="xsql")
                nc.scalar.dma_start(out=xsql_b[:], in_=xsql_d[b])
                xsql_bs[b] = xsql_b
                psum_es[b] = tuple(
                    ps_e.tile([K, D + 1], f32, tag=f"pe{pp}", name=f"psum_e{pp}_b{b}")
                    for pp in range(NACC)
                )
                first_mm2[b] = [True] * NACC
            soff = sib * SUPN
            x_chs = []
            for ch in range(NCH):
                x_ch = xpool.tile([D, CHN], f8, tag=f"xch{ch}")
                nc.sync.dma_start(
                    out=x_ch[:],
                    in_=x_d[b][:, soff + CHN * ch : soff + CHN * (ch + 1)],
                )
                x_chs.append(x_ch)
            xt_sb = xtpool.tile([128, TPS, D + 1], f8)
            nc.scalar.dma_start(
                out=xt_sb[:], in_=xt_d[b][:, sib * TPS : (sib + 1) * TPS, :]
            )
            psum_xc = ps_xc.tile([128, TPS * K], f32)
            for h in range(2):
                nc.tensor.matmul(
                    psum_xc[:, 512 * h : 512 * (h + 1)],
                    lhsT=xsql_bs[b][:SEEDR, sib, :],
                    rhs=wdiag_sb[:, 512 * h : 512 * (h + 1)],
                    start=True,
                    stop=False,
                    skip_group_check=True,
                )
            for i in range(TPS):
                nc.tensor.matmul(
                    psum_xc[:, K * i : K * (i + 1)],
                    lhsT=x_chs[i // (TPS // NCH)][
                        :, TILN * (i % (TPS // NCH)) : TILN * (i % (TPS // NCH) + 1)
                    ],
                    rhs=w1_sb[:, :],
                    start=False,
                    stop=True,
                    skip_group_check=True,
                )

            p_sb = ppool.tile([128, TPS * K], bf, tag="pexp")
            nc.scalar.activation(
                p_sb[:], psum_xc[:], mybir.ActivationFunctionType.Exp,
                scale=1.0 / 64.0,
            )
            p3 = p_sb[:].rearrange("p (i k) -> p i k", k=K)
            s_sb = vpool.tile([128, TPS], f32, tag="s")
            nc.vector.reduce_sum(s_sb[:], p3, axis=mybir.AxisListType.X)
            sinv_sb = vpool.tile([128, TPS], f32, tag="sinv")
            nc.vector.reciprocal(sinv_sb[:], s_sb[:])
            pn_sb = npool.tile([128, TPS * K], bf, tag="pn")
            norm_eng = nc.vector if sib == 0 else nc.gpsimd
            norm_eng.tensor_tensor(
                pn_sb[:].rearrange("p (i k) -> p i k", k=K),
                p3,
                sinv_sb[:].broadcast_to([128, TPS, K]),
                op=mybir.AluOpType.mult,
            )

            pending.append((b, sib, pn_sb, xt_sb))
            if len(pending) > 1:
                emit_mm2s(*pending.pop(0))

        while pending:
            emit_mm2s(*pending.pop(0))

    nc.compile()
    return nc


def _get_module():
    if "nc" not in _CACHE:
        _CACHE["nc"] = _build_module()
    return _CACHE["nc"]


def _host_prep(x, codewords, scale):
    x = np.asarray(x, dtype=np.float32)
    c = np.asarray(codewords, dtype=np.float32)
    s = np.asarray(scale, dtype=np.float32)

    w = -s                           # (K,) in (0, 1)
    maxs = float(w.max())
    wm = w - maxs                                           # (K,) <= 0
    wh = wm.astype(BF16).astype(np.float32)
    wl = (wm - wh).astype(BF16).astype(np.float32)
    wcsq = w * (c * c).sum(axis=1)                          # (K,)
    w1 = (-128.0 * (w[:, None] * c)).T.astype(F8E3)         # (D, K) = 64 * (-2 w c)

    xf = x.reshape(B, D, N)
    xsq = np.einsum("bdn,bdn->bn", xf, xf)                  # (B, N) fp32
    m = xsq.astype(BF16).astype(np.float32)
    r = (xsq - m).astype(BF16).astype(np.float32)
    # xsql[b, row, sb, p]; row j' in 0..31 -> m of n-subtile j', 32..63 -> r,
    # 64..95 -> m again (for the wl term), 96 -> ones (wcsq term), 97.. -> pad
    # n = sb*SUPN + j'*128 + p
    mt = m.reshape(B, NSB, TPS, 128).transpose(0, 2, 1, 3)  # (B, j', sb, p)
    rt = r.reshape(B, NSB, TPS, 128).transpose(0, 2, 1, 3)
    xsql = np.zeros((B, 128, NSB, 128), dtype=BF16)
    xsql[:, 0:32] = mt.astype(BF16)
    xsql[:, 32:64] = rt.astype(BF16)
    xsql[:, 64:96] = mt.astype(BF16)
    xsql[:, 96] = 1.0

    wdiag = np.zeros((SEEDR, TPS, K), dtype=np.float32)
    jj = np.arange(TPS)
    wdiag[jj, jj, :] = 64.0 * wh[None, :]
    wdiag[32 + jj, jj, :] = 64.0 * wh[None, :]
    wdiag[64 + jj, jj, :] = 64.0 * wl[None, :]
    wdiag[96, :, :] = 64.0 * wcsq[None, :]
    wdiag = wdiag.reshape(SEEDR, TPS * K).astype(BF16)

    x8 = xf.astype(F8E3)                                    # (B, D, N)
    # xt[b, p, gi, d] = xf[b, d, gi*128 + p];  xt[..., D] = 1.0
    xt = np.ones((B, N // TILN, TILN, D + 1), dtype=F8E3)
    xt[:, :, :, :D] = xf.transpose(0, 2, 1).reshape(B, N // TILN, TILN, D).astype(F8E3)
    xt = np.ascontiguousarray(xt.transpose(0, 2, 1, 3))     # (B, 128, N/128, D+1)
    return x8, xt, xsql, wdiag, w1


def make_in_maps(x, codewords, scale):
    x8, xt, xsql, wdiag, w1 = _host_prep(x, codewords, scale)
    in_maps = []
    for ci in range(NCORES):
        sl = slice(BPC * ci, BPC * (ci + 1))
        in_maps.append(
            {
                "x": np.ascontiguousarray(x8[sl]),
                "xt": np.ascontiguousarray(xt[sl]),
                "xsql": np.ascontiguousarray(xsql[sl]),
                "wdiag": wdiag,
                "w1": w1,
            }
        )
    return in_maps


def finish_output(results, codewords):
    c = np.asarray(codewords, dtype=np.float32)
    out = np.zeros((B, K * D), dtype=np.float32)
    for ci, r in enumerate(results):
        for bb in range(BPC):
            e_parts = r["out_e"][bb].sum(axis=1)            # (K, D+1)
            e = e_parts[:, :D] - e_parts[:, D : D + 1] * c
            out[BPC * ci + bb] = e.reshape(-1)
    return out


def kernel(x, codewords, scale):
    from concourse.bass_utils import run_bass_kernel_spmd
    from concourse.bass_interp import get_hw_module

    nc = _get_module()
    in_maps = make_in_maps(x, codewords, scale)

    old_m = nc.m
    nc.m = get_hw_module(nc.m)
    try:
        res = run_bass_kernel_spmd(nc, in_maps, core_ids=list(range(NCORES)))
    finally:
        nc.m = old_m
    return finish_output(res.results, codewords)


# revision 11
# speedup vs baseline: 1.4432x; 1.1732x over previous
"""DeepTEN encoding kernel for Trainium2 (8 NeuronCores, SPMD data-parallel over batch).

Math (per batch b):
    xf = x[b] viewed (D, N), N = H*W
    logits[n,k] = w_k * dist[n,k],  w = -scale > 0 ;  A = softmax_k(logits)
    E[k,d] = sum_n A[n,k] * (xf[d,n] - c[k,d]) = (A^T X)[k,d] - colsum(A)[k]*c[k,d]

Device decomposition (softmax in (n-partitions, k-free) layout, PSUM holds the
FULL shifted exponent so a single EXP activation yields the softmax numerator):
    PSUM[n,(j,k)]  = seed + mm1, everything prescaled by 64:
      seed (one [97,128]x[97,1024] bf16 matmul per superblock, split in 2 PSUM
      banks) adds 64*[ (w_k-maxs)*xsq_n + w_k*csq_k ] using an exact hi/lo
      split: wm*xsq ~= wh*m + wh*r + wl*m with wh=bf16(wm), wl=bf16(wm-wh),
      m=bf16(xsq), r=bf16(xsq-m), laid out block-diagonally over n-subtiles j'.
      mm1 (fp8 e3m4 x-tile stationary, fp8 w1=-128*w*c moving) adds 64*(-2w<x,c>).
    P = exp(PSUM/64) via one ACT instruction (scale=1/64), S = rowsum_k, A = P/S.
    Aggregation is FLIPPED vs the textbook A^T X: per 128-n tile the fp8
    xt-tile is the (cheap-to-load) stationary and the A-slice streams:
      psum_ET[d, k] += sum_n xt[n,d] * A[n,k]     (out free = 32, not 129)
      psum_cs[0, k] += sum_n 1 * A[n,k]           (ones stationary, colsum)
    both accumulate whole-batch in PSUM with a 2-deep accumulator rotation.

x is uploaded twice in fp8 e3m4 — (D,N) for the distance matmuls and
pre-transposed tiles (p, gi, d) for the aggregation matmuls — so total HBM
traffic ~= 17.4 MB/core (vs 33.5 MB for the bf16 dual upload). e3m4 keeps
4 mantissa bits; numpy sim of this exact quantization gives maxabsrel ~1.1e-2
vs the 2e-2 gate. A stays bf16 (fp8 A fails the gate). The mm2s of
superblock s are emitted after the softmax chain of superblock s+2
(2-deep software pipelining) so the PE never waits on the DVE/ACT chain.
"""
import os
import sys
import numpy as np

sys.path.insert(0, "/opt/trn_rl_repo")

import ml_dtypes  # noqa: E402

BF16 = ml_dtypes.bfloat16
F8E3 = ml_dtypes.float8_e3m4

B, D, H, W = 32, 128, 128, 128
K = 32
N = H * W            # 16384
NCORES = 8
BPC = B // NCORES    # batches per core
TILN = 128           # n per tile (matmul stationary width)
TPS = 32             # tiles per superblock
SUPN = TILN * TPS    # 4096 n per superblock
NSB = N // SUPN      # 4 superblocks per batch
SEEDR = 97           # seed lhsT rows: 32 m + 32 r + 32 m + 1 ones
LAG = 2              # superblocks between softmax emit and its mm2s

_CACHE = {}


def _build_module():
    from contextlib import ExitStack
    import concourse.tile as tile
    from concourse import bacc, mybir

    nc = bacc.Bacc("TRN2", target_bir_lowering=False, debug=False, num_devices=NCORES)
    bf = mybir.dt.bfloat16
    f8 = mybir.dt.float8e3
    f32 = mybir.dt.float32

    x_d = nc.dram_tensor("x", [BPC, D, N], f8, kind="ExternalInput").ap()
    # xt[b, p, gi, d] = x[b, d, gi*128 + p]
    xt_d = nc.dram_tensor("xt", [BPC, 128, N // TILN, D], f8, kind="ExternalInput").ap()
    # seed lhsT rows (see module docstring); xsql[b, row, sb, p], rows 97..127 pad
    xsql_d = nc.dram_tensor("xsql", [BPC, 128, NSB, 128], bf, kind="ExternalInput").ap()
    # seed rhs: wdiag[row, (j, k)] block-diagonal over n-subtiles j
    wdiag_d = nc.dram_tensor("wdiag", [128, TPS * K], bf, kind="ExternalInput").ap()
    w1_d = nc.dram_tensor("w1", [D, K], f8, kind="ExternalInput").ap()
    oute_d = nc.dram_tensor("out_e", [BPC, D, 2, K], f32, kind="ExternalOutput").ap()
    outcs_d = nc.dram_tensor("out_cs", [BPC, 1, 2, K], f32, kind="ExternalOutput").ap()

    with tile.TileContext(nc) as tc, ExitStack() as ctx:
        cpool = ctx.enter_context(tc.tile_pool(name="const", bufs=1))
        xpool = ctx.enter_context(tc.tile_pool(name="xblk", bufs=3))
        xtpool = ctx.enter_context(tc.tile_pool(name="xtblk", bufs=LAG + 2))
        qpool = ctx.enter_context(tc.tile_pool(name="xsqb", bufs=2))
        ppool = ctx.enter_context(tc.tile_pool(name="pexp", bufs=3))
        npool = ctx.enter_context(tc.tile_pool(name="pnorm", bufs=LAG + 2))
        vpool = ctx.enter_context(tc.tile_pool(name="small", bufs=4))
        ps_xc = ctx.enter_context(tc.tile_pool(name="ps_xc", bufs=2, space="PSUM"))
        ps_e = ctx.enter_context(tc.tile_pool(name="ps_e", bufs=1, space="PSUM"))
        ps_cs = ctx.enter_context(tc.tile_pool(name="ps_cs", bufs=1, space="PSUM"))

        w1_sb = cpool.tile([D, K], f8)
        nc.scalar.dma_start(out=w1_sb[:], in_=w1_d[:, :])
        wdiag_sb = cpool.tile([128, TPS * K], bf)
        nc.scalar.dma_start(out=wdiag_sb[:], in_=wdiag_d[:, :])
        ones_sb = cpool.tile([128, 1], bf)
        nc.vector.memset(ones_sb[:], 1.0)

        # Software pipeline: mm2s of superblock s are emitted after the
        # softmax chain of superblock s+LAG, so the PE hides the chain latency.
        pending = []  # (b, sib, pn_sb, xt_sb)
        psum_es = {}
        xsql_bs = {}
        first_mm2 = {}

        def emit_mm2s(b, sib, pn_sb, xt_sb):
            pes, pcs = psum_es[b]
            ff_et, ff_cs = first_mm2[b]
            last = sib == NSB - 1
            for i in range(TPS):
                pp = i % 2
                nc.tensor.matmul(
                    pes[pp][:],
                    lhsT=xt_sb[:, i, :],
                    rhs=pn_sb[:, K * i : K * (i + 1)],
                    start=ff_et[pp],
                    stop=(last and i >= TPS - 2),
                    skip_group_check=True,
                )
                ff_et[pp] = False
            for i in range(TPS):
                pp = i % 2
                nc.tensor.matmul(
                    pcs[pp][:],
                    lhsT=ones_sb[:],
                    rhs=pn_sb[:, K * i : K * (i + 1)],
                    start=ff_cs[pp],
                    stop=(last and i >= TPS - 2),
                    skip_group_check=True,
                )
                ff_cs[pp] = False
            if last:
                e_sb = vpool.tile([D, 2, K], f32, tag="e_out")
                cs_sb = vpool.tile([1, 2, K], f32, tag="cs_out")
                for pp in range(2):
                    nc.vector.tensor_copy(e_sb[:, pp, :], pes[pp][:])
                    nc.vector.tensor_copy(cs_sb[:, pp, :], pcs[pp][:])
                nc.sync.dma_start(out=oute_d[b], in_=e_sb[:])
                nc.sync.dma_start(out=outcs_d[b], in_=cs_sb[:])

        for gsup in range(BPC * NSB):
            b, sib = divmod(gsup, NSB)
            if sib == 0:
                xsql_b = qpool.tile([128, NSB, 128], bf, tag="xsql")
                nc.scalar.dma_start(out=xsql_b[:], in_=xsql_d[b])
                xsql_bs[b] = xsql_b
                psum_es[b] = (
                    tuple(
                        ps_e.tile([128, K], f32, tag=f"pe{pp}", name=f"psum_e{pp}_b{b}")
                        for pp in range(2)
                    ),
                    tuple(
                        ps_cs.tile([1, K], f32, tag=f"cs{pp}", name=f"psum_cs{pp}_b{b}")
                        for pp in range(2)
                    ),
                )
                first_mm2[b] = ([True, True], [True, True])
            soff = sib * SUPN
            x_sb = xpool.tile([D, SUPN], f8)
            nc.sync.dma_start(out=x_sb[:], in_=x_d[b][:, soff : soff + SUPN])
            xt_sb = xtpool.tile([128, TPS, D], f8)
            nc.scalar.dma_start(
                out=xt_sb[:], in_=xt_d[b][:, sib * TPS : (sib + 1) * TPS, :]
            )
            psum_xc = ps_xc.tile([128, TPS * K], f32)
            for h in range(2):
                nc.tensor.matmul(
                    psum_xc[:, 512 * h : 512 * (h + 1)],
                    lhsT=xsql_bs[b][:SEEDR, sib, :],
                    rhs=wdiag_sb[:SEEDR, 512 * h : 512 * (h + 1)],
                    start=True,
                    stop=False,
                    skip_group_check=True,
                )
            for i in range(TPS):
                nc.tensor.matmul(
                    psum_xc[:, K * i : K * (i + 1)],
                    lhsT=x_sb[:, TILN * i : TILN * (i + 1)],
                    rhs=w1_sb[:, :],
                    start=False,
                    stop=True,
                    skip_group_check=True,
                )

            p_sb = ppool.tile([128, TPS * K], bf, tag="pexp")
            nc.scalar.activation(
                p_sb[:], psum_xc[:], mybir.ActivationFunctionType.Exp,
                scale=1.0 / 64.0,
            )
            p3 = p_sb[:].rearrange("p (i k) -> p i k", k=K)
            s_sb = vpool.tile([128, TPS], f32, tag="s")
            nc.vector.reduce_sum(s_sb[:], p3, axis=mybir.AxisListType.X)
            sinv_sb = vpool.tile([128, TPS], f32, tag="sinv")
            nc.vector.reciprocal(sinv_sb[:], s_sb[:])
            pn_sb = npool.tile([128, TPS * K], bf, tag="pn")
            norm_eng = nc.vector if sib == 0 else nc.gpsimd
            norm_eng.tensor_tensor(
                pn_sb[:].rearrange("p (i k) -> p i k", k=K),
                p3,
                sinv_sb[:].broadcast_to([128, TPS, K]),
                op=mybir.AluOpType.mult,
            )

            pending.append((b, sib, pn_sb, xt_sb))
            if len(pending) > LAG:
                emit_mm2s(*pending.pop(0))

        while pending:
            emit_mm2s(*pending.pop(0))

    nc.compile()
    return nc


def _get_module():
    if "nc" not in _CACHE:
        _CACHE["nc"] = _build_module()
    return _CACHE["nc"]


def _host_prep(x, codewords, scale):
    x = np.asarray(x, dtype=np.float32)
    c = np.asarray(codewords, dtype=np.float32)
    s = np.asarray(scale, dtype=np.float32)

    w = -s                           # (K,) in (0, 1)
    maxs = float(w.max())
    wm = w - maxs                                           # (K,) <= 0
    wh = wm.astype(BF16).astype(np.float32)
    wl = (wm - wh).astype(BF16).astype(np.float32)
    wcsq = w * (c * c).sum(axis=1)                          # (K,)
    w1 = (-128.0 * (w[:, None] * c)).T.astype(F8E3)         # (D, K) = 64 * (-2 w c)

    xf = x.reshape(B, D, N)
    xsq = np.einsum("bdn,bdn->bn", xf, xf)                  # (B, N) fp32
    m = xsq.astype(BF16).astype(np.float32)
    r = (xsq - m).astype(BF16).astype(np.float32)
    # xsql[b, row, sb, p]; row j' in 0..31 -> m of n-subtile j', 32..63 -> r,
    # 64..95 -> m again (for the wl term), 96 -> ones (wcsq term), 97.. -> pad
    # n = sb*SUPN + j'*128 + p
    mt = m.reshape(B, NSB, TPS, 128).transpose(0, 2, 1, 3)  # (B, j', sb, p)
    rt = r.reshape(B, NSB, TPS, 128).transpose(0, 2, 1, 3)
    xsql = np.zeros((B, 128, NSB, 128), dtype=BF16)
    xsql[:, 0:32] = mt.astype(BF16)
    xsql[:, 32:64] = rt.astype(BF16)
    xsql[:, 64:96] = mt.astype(BF16)
    xsql[:, 96] = 1.0

    wdiag = np.zeros((128, TPS, K), dtype=np.float32)
    jj = np.arange(TPS)
    wdiag[jj, jj, :] = 64.0 * wh[None, :]
    wdiag[32 + jj, jj, :] = 64.0 * wh[None, :]
    wdiag[64 + jj, jj, :] = 64.0 * wl[None, :]
    wdiag[96, :, :] = 64.0 * wcsq[None, :]
    wdiag = wdiag.reshape(128, TPS * K).astype(BF16)

    x8 = xf.astype(F8E3)                                    # (B, D, N)
    # xt[b, p, gi, d] = xf[b, d, gi*128 + p]
    xt = np.ascontiguousarray(
        xf.transpose(0, 2, 1).reshape(B, N // TILN, TILN, D).transpose(0, 2, 1, 3)
        .astype(F8E3)
    )                                                       # (B, 128, N/128, D)
    return x8, xt, xsql, wdiag, w1


def make_in_maps(x, codewords, scale):
    x8, xt, xsql, wdiag, w1 = _host_prep(x, codewords, scale)
    in_maps = []
    for ci in range(NCORES):
        sl = slice(BPC * ci, BPC * (ci + 1))
        in_maps.append(
            {
                "x": np.ascontiguousarray(x8[sl]),
                "xt": np.ascontiguousarray(xt[sl]),
                "xsql": np.ascontiguousarray(xsql[sl]),
                "wdiag": wdiag,
                "w1": w1,
            }
        )
    return in_maps


def finish_output(results, codewords):
    c = np.asarray(codewords, dtype=np.float32)
    out = np.zeros((B, K * D), dtype=np.float32)
    for ci, r in enumerate(results):
        for bb in range(BPC):
            oe = r["out_e"][bb]                              # (D, 2, K)
            et = oe[:, 0, :] + oe[:, 1, :]                   # (D, K)
            ocs = r["out_cs"][bb][0]                         # (2, K)
            cs = ocs[0] + ocs[1]                             # (K,)
            e = et.T - cs[:, None] * c
            out[BPC * ci + bb] = e.reshape(-1)
    return out


def kernel(x, codewords, scale):
    from concourse.bass_utils import run_bass_kernel_spmd
    from concourse.bass_interp import get_hw_module

    nc = _get_module()
    in_maps = make_in_maps(x, codewords, scale)

    old_m = nc.m
    nc.m = get_hw_module(nc.m)
    try:
        res = run_bass_kernel_spmd(nc, in_maps, core_ids=list(range(NCORES)))
    finally:
        nc.m = old_m
    return finish_output(res.results, codewords)


# revision 12
# speedup vs baseline: 1.6122x; 1.1171x over previous
"""DeepTEN encoding kernel for Trainium2 (8 NeuronCores, SPMD data-parallel over batch).

Math (per batch b):
    xf = x[b] viewed (D, N), N = H*W
    logits[n,k] = w_k * dist[n,k],  w = -scale > 0 ;  A = softmax_k(logits)
    E[k,d] = sum_n A[n,k] * (xf[d,n] - c[k,d]) = (A^T X)[k,d] - colsum(A)[k]*c[k,d]

Device decomposition (softmax in (n-partitions, k-free) layout, PSUM holds the
FULL shifted exponent so a single EXP activation yields the softmax numerator):
    PSUM[n,(j,k)]  = seed + mm1, everything prescaled by 64:
      seed (one [97,128]x[97,1024] bf16 matmul per superblock, split in 2 PSUM
      banks) adds 64*[ (w_k-maxs)*xsq_n + w_k*csq_k ] using an exact hi/lo
      split: wm*xsq ~= wh*m + wh*r + wl*m with wh=bf16(wm), wl=bf16(wm-wh),
      m=bf16(xsq), r=bf16(xsq-m), laid out block-diagonally over n-subtiles j'.
      mm1 (fp8 e3m4 x-tile stationary, fp8 w1=-128*w*c moving) adds 64*(-2w<x,c>).
    P = exp(PSUM/64) via one ACT instruction (scale=1/64), S = rowsum_k, A = P/S.
    Aggregation is FLIPPED vs the textbook A^T X: per 128-n tile the fp8
    xt-tile is the (cheap-to-load) stationary and the A-slice streams:
      psum_ET[d, k] += sum_n xt[n,d] * A[n,k]     (out free = 32, not 129)
      psum_cs[0, k] += sum_n 1 * A[n,k]           (ones stationary, colsum)
    both accumulate whole-batch in PSUM with a 2-deep accumulator rotation.

x is uploaded twice in fp8 e3m4 — (D,N) for the distance matmuls and
pre-transposed tiles (p, gi, d) for the aggregation matmuls — so total HBM
traffic ~= 17.4 MB/core (vs 33.5 MB for the bf16 dual upload). e3m4 keeps
4 mantissa bits; numpy sim of this exact quantization gives maxabsrel ~1.1e-2
vs the 2e-2 gate. A stays bf16 (fp8 A fails the gate). The mm2s of
superblock s are emitted after the softmax chain of superblock s+2
(2-deep software pipelining) so the PE never waits on the DVE/ACT chain.
"""
import os
import sys
import numpy as np

sys.path.insert(0, "/opt/trn_rl_repo")

import ml_dtypes  # noqa: E402

BF16 = ml_dtypes.bfloat16
F8E3 = ml_dtypes.float8_e3m4

B, D, H, W = 32, 128, 128, 128
K = 32
N = H * W            # 16384
NCORES = 8
BPC = B // NCORES    # batches per core
TILN = 128           # n per tile (matmul stationary width)
TPS = 32             # tiles per superblock
SUPN = TILN * TPS    # 4096 n per superblock
NSB = N // SUPN      # 4 superblocks per batch
SEEDR = 97           # seed lhsT rows: 32 m + 32 r + 32 m + 1 ones
LAG = 2              # superblocks between softmax emit and its mm2s

_CACHE = {}


def _build_module():
    from contextlib import ExitStack
    import concourse.tile as tile
    from concourse import bacc, mybir

    nc = bacc.Bacc("TRN2", target_bir_lowering=False, debug=False, num_devices=NCORES)
    bf = mybir.dt.bfloat16
    f8 = mybir.dt.float8e3
    f32 = mybir.dt.float32

    x_d = nc.dram_tensor("x", [BPC, D, N], f8, kind="ExternalInput").ap()
    # xt[b, p, gi, d] = x[b, d, gi*128 + p]
    xt_d = nc.dram_tensor("xt", [BPC, 128, N // TILN, D], f8, kind="ExternalInput").ap()
    # seed lhsT rows (see module docstring); xsql[b, row, sb, p], rows 97..127 pad
    xsql_d = nc.dram_tensor("xsql", [BPC, 128, NSB, 128], bf, kind="ExternalInput").ap()
    # seed rhs: wdiag[row, (j, k)] block-diagonal over n-subtiles j
    wdiag_d = nc.dram_tensor("wdiag", [128, TPS * K], bf, kind="ExternalInput").ap()
    w1_d = nc.dram_tensor("w1", [D, K], f8, kind="ExternalInput").ap()
    oute_d = nc.dram_tensor("out_e", [BPC, D, 2, K], f32, kind="ExternalOutput").ap()
    outcs_d = nc.dram_tensor("out_cs", [BPC, 1, TPS * K], f32, kind="ExternalOutput").ap()

    with tile.TileContext(nc) as tc, ExitStack() as ctx:
        cpool = ctx.enter_context(tc.tile_pool(name="const", bufs=1))
        xpool = ctx.enter_context(tc.tile_pool(name="xblk", bufs=3))
        xtpool = ctx.enter_context(tc.tile_pool(name="xtblk", bufs=LAG + 2))
        qpool = ctx.enter_context(tc.tile_pool(name="xsqb", bufs=2))
        ppool = ctx.enter_context(tc.tile_pool(name="pexp", bufs=3))
        npool = ctx.enter_context(tc.tile_pool(name="pnorm", bufs=LAG + 2))
        vpool = ctx.enter_context(tc.tile_pool(name="small", bufs=4))
        ps_xc = ctx.enter_context(tc.tile_pool(name="ps_xc", bufs=2, space="PSUM"))
        ps_e = ctx.enter_context(tc.tile_pool(name="ps_e", bufs=1, space="PSUM"))
        ps_cs = ctx.enter_context(tc.tile_pool(name="ps_cs", bufs=1, space="PSUM"))

        wdiag_sb = cpool.tile([128, TPS * K], bf)
        nc.sync.dma_start(out=wdiag_sb[:], in_=wdiag_d[:, :])
        w1_sb = cpool.tile([D, K], f8)
        nc.scalar.dma_start(out=w1_sb[:], in_=w1_d[:, :])
        ones_sb = cpool.tile([128, 1], bf)
        nc.vector.memset(ones_sb[:], 1.0)

        # Software pipeline: mm2s of superblock s are emitted after the
        # softmax chain of superblock s+LAG, so the PE hides the chain latency.
        pending = []  # (b, sib, pn_sb, xt_sb)
        psum_es = {}
        xsql_bs = {}
        first_mm2 = {}

        def emit_mm2s(b, sib, pn_sb, xt_sb):
            pes, pcs = psum_es[b]
            ff_et, ff_cs = first_mm2[b]
            last = sib == NSB - 1
            for i in range(TPS):
                pp = i % 2
                nc.tensor.matmul(
                    pes[pp][:],
                    lhsT=xt_sb[:, i, :],
                    rhs=pn_sb[:, K * i : K * (i + 1)],
                    start=ff_et[pp],
                    stop=(last and i >= TPS - 2),
                    skip_group_check=True,
                )
                ff_et[pp] = False
            for h in range(2):
                nc.tensor.matmul(
                    pcs[0:1, 512 * h : 512 * (h + 1)],
                    lhsT=ones_sb[:],
                    rhs=pn_sb[:, 512 * h : 512 * (h + 1)],
                    start=ff_cs[h],
                    stop=last,
                    skip_group_check=True,
                )
                ff_cs[h] = False
            if last:
                e_sb = vpool.tile([D, 2, K], f32, tag="e_out")
                cs_sb = vpool.tile([1, TPS * K], f32, tag="cs_out")
                for pp in range(2):
                    nc.vector.tensor_copy(e_sb[:, pp, :], pes[pp][:])
                nc.vector.tensor_copy(cs_sb[:], pcs[:])
                nc.sync.dma_start(out=oute_d[b], in_=e_sb[:])
                nc.sync.dma_start(out=outcs_d[b], in_=cs_sb[:])

        def load_xsql(b):
            xsql_b = qpool.tile([128, NSB, 128], bf, tag="xsql")
            nc.sync.dma_start(out=xsql_b[:], in_=xsql_d[b])
            xsql_bs[b] = xsql_b

        load_xsql(0)
        for gsup in range(BPC * NSB):
            b, sib = divmod(gsup, NSB)
            if sib == 1 and b + 1 < BPC:
                load_xsql(b + 1)
            if sib == 0:
                psum_es[b] = (
                    tuple(
                        ps_e.tile([128, K], f32, tag=f"pe{pp}", name=f"psum_e{pp}_b{b}")
                        for pp in range(2)
                    ),
                    ps_cs.tile([1, TPS * K], f32, tag="cs", name=f"psum_cs_b{b}"),
                )
                first_mm2[b] = ([True, True], [True, True])
            soff = sib * SUPN
            x_sb = xpool.tile([D, SUPN], f8)
            nc.sync.dma_start(out=x_sb[:], in_=x_d[b][:, soff : soff + SUPN])
            xt_sb = xtpool.tile([128, TPS, D], f8)
            nc.scalar.dma_start(
                out=xt_sb[:], in_=xt_d[b][:, sib * TPS : (sib + 1) * TPS, :]
            )
            psum_xc = ps_xc.tile([128, TPS * K], f32)
            for h in range(2):
                nc.tensor.matmul(
                    psum_xc[:, 512 * h : 512 * (h + 1)],
                    lhsT=xsql_bs[b][:SEEDR, sib, :],
                    rhs=wdiag_sb[:SEEDR, 512 * h : 512 * (h + 1)],
                    start=True,
                    stop=False,
                    skip_group_check=True,
                )
            for i in range(TPS):
                nc.tensor.matmul(
                    psum_xc[:, K * i : K * (i + 1)],
                    lhsT=x_sb[:, TILN * i : TILN * (i + 1)],
                    rhs=w1_sb[:, :],
                    start=False,
                    stop=True,
                    skip_group_check=True,
                )

            p_sb = ppool.tile([128, TPS * K], bf, tag="pexp")
            nc.scalar.activation(
                p_sb[:], psum_xc[:], mybir.ActivationFunctionType.Exp,
                scale=1.0 / 64.0,
            )
            p3 = p_sb[:].rearrange("p (i k) -> p i k", k=K)
            s_sb = vpool.tile([128, TPS], f32, tag="s")
            nc.vector.reduce_sum(s_sb[:], p3, axis=mybir.AxisListType.X)
            sinv_sb = vpool.tile([128, TPS], f32, tag="sinv")
            nc.vector.reciprocal(sinv_sb[:], s_sb[:])
            pn_sb = npool.tile([128, TPS * K], bf, tag="pn")
            hj = TPS // 2
            nc.gpsimd.tensor_tensor(
                pn_sb[:, : hj * K].rearrange("p (i k) -> p i k", k=K),
                p3[:, :hj, :],
                sinv_sb[:, :hj].broadcast_to([128, hj, K]),
                op=mybir.AluOpType.mult,
            )
            nc.vector.tensor_tensor(
                pn_sb[:, hj * K :].rearrange("p (i k) -> p i k", k=K),
                p3[:, hj:, :],
                sinv_sb[:, hj:].broadcast_to([128, hj, K]),
                op=mybir.AluOpType.mult,
            )

            pending.append((b, sib, pn_sb, xt_sb))
            if len(pending) > LAG:
                emit_mm2s(*pending.pop(0))

        while pending:
            emit_mm2s(*pending.pop(0))

    nc.compile()
    return nc


def _get_module():
    if "nc" not in _CACHE:
        _CACHE["nc"] = _build_module()
    return _CACHE["nc"]


def _host_prep(x, codewords, scale):
    x = np.asarray(x, dtype=np.float32)
    c = np.asarray(codewords, dtype=np.float32)
    s = np.asarray(scale, dtype=np.float32)

    w = -s                           # (K,) in (0, 1)
    maxs = float(w.max())
    wm = w - maxs                                           # (K,) <= 0
    wh = wm.astype(BF16).astype(np.float32)
    wl = (wm - wh).astype(BF16).astype(np.float32)
    wcsq = w * (c * c).sum(axis=1)                          # (K,)
    w1 = (-128.0 * (w[:, None] * c)).T.astype(F8E3)         # (D, K) = 64 * (-2 w c)

    xf = x.reshape(B, D, N)
    xsq = np.einsum("bdn,bdn->bn", xf, xf)                  # (B, N) fp32
    m = xsq.astype(BF16).astype(np.float32)
    r = (xsq - m).astype(BF16).astype(np.float32)
    # xsql[b, row, sb, p]; row j' in 0..31 -> m of n-subtile j', 32..63 -> r,
    # 64..95 -> m again (for the wl term), 96 -> ones (wcsq term), 97.. -> pad
    # n = sb*SUPN + j'*128 + p
    mt = m.reshape(B, NSB, TPS, 128).transpose(0, 2, 1, 3)  # (B, j', sb, p)
    rt = r.reshape(B, NSB, TPS, 128).transpose(0, 2, 1, 3)
    xsql = np.zeros((B, 128, NSB, 128), dtype=BF16)
    xsql[:, 0:32] = mt.astype(BF16)
    xsql[:, 32:64] = rt.astype(BF16)
    xsql[:, 64:96] = mt.astype(BF16)
    xsql[:, 96] = 1.0

    wdiag = np.zeros((128, TPS, K), dtype=np.float32)
    jj = np.arange(TPS)
    wdiag[jj, jj, :] = 64.0 * wh[None, :]
    wdiag[32 + jj, jj, :] = 64.0 * wh[None, :]
    wdiag[64 + jj, jj, :] = 64.0 * wl[None, :]
    wdiag[96, :, :] = 64.0 * wcsq[None, :]
    wdiag = wdiag.reshape(128, TPS * K).astype(BF16)

    x8 = xf.astype(F8E3)                                    # (B, D, N)
    # xt[b, p, gi, d] = xf[b, d, gi*128 + p]
    xt = np.ascontiguousarray(
        xf.transpose(0, 2, 1).reshape(B, N // TILN, TILN, D).transpose(0, 2, 1, 3)
        .astype(F8E3)
    )                                                       # (B, 128, N/128, D)
    return x8, xt, xsql, wdiag, w1


def make_in_maps(x, codewords, scale):
    x8, xt, xsql, wdiag, w1 = _host_prep(x, codewords, scale)
    in_maps = []
    for ci in range(NCORES):
        sl = slice(BPC * ci, BPC * (ci + 1))
        in_maps.append(
            {
                "x": np.ascontiguousarray(x8[sl]),
                "xt": np.ascontiguousarray(xt[sl]),
                "xsql": np.ascontiguousarray(xsql[sl]),
                "wdiag": wdiag,
                "w1": w1,
            }
        )
    return in_maps


def finish_output(results, codewords):
    c = np.asarray(codewords, dtype=np.float32)
    out = np.zeros((B, K * D), dtype=np.float32)
    for ci, r in enumerate(results):
        for bb in range(BPC):
            oe = r["out_e"][bb]                              # (D, 2, K)
            et = oe[:, 0, :] + oe[:, 1, :]                   # (D, K)
            cs = r["out_cs"][bb][0].reshape(TPS, K).sum(0)   # (K,)
            e = et.T - cs[:, None] * c
            out[BPC * ci + bb] = e.reshape(-1)
    return out


def kernel(x, codewords, scale):
    from concourse.bass_utils import run_bass_kernel_spmd
    from concourse.bass_interp import get_hw_module

    nc = _get_module()
    in_maps = make_in_maps(x, codewords, scale)

    old_m = nc.m
    nc.m = get_hw_module(nc.m)
    try:
        res = run_bass_kernel_spmd(nc, in_maps, core_ids=list(range(NCORES)))
    finally:
        nc.m = old_m
    return finish_output(res.results, codewords)
